# revision 4
# baseline (speedup 1.0000x reference)
"""Trainium2 Bass kernel v3 for EnhancedLocalComplexAttention.

v3 vs v2:
  - o-projection in bf16 (v2 used f32r: HW f32r matmul has ~1.6e-2 rms error
    and dominated the error budget; bf16 drops total rel-err to ~5e-3).
  - Karatsuba (3M) complex matmuls for q/k/v/o projections: for y = x @ W^T
    with A = Wre^T, B = Wim^T:
      t1 = xr@(A+B), t2 = xi@(A-B), t3 = (xr+xi)@B
      y_re = t1 - t3, y_im = t2 + t3
    -> 12 matmuls per 128-feature block instead of 16 (-25% PE on
    projections). t3 is copied PSUM->SBUF bf16 (Act), the two combines are
    DVE tensor_tensor ops (PSUM + SBUF operand) replacing the plain copies.
  - q/k combines are half-width (64-partition) ops with 32-aligned partition
    base shifts so qcat keeps the v2 interleaved (re|im)-per-head layout and
    the attention inner loop is unchanged.
  - ao is stored as separate ao_re/ao_im/ao_sum pair-packed tiles (bf16) for
    the Karatsuba o-projection; pv PSUM drains as two half-width Act copies,
    ao_sum = ao_re + ao_im on Pool (SBUF-only).
  - Softmax row-sum + attn scaling moved to Pool to unload DVE.
  - x_sum = bf16(x_re + x_im) precomputed on host (extra 0.5MB DMA/batch).
"""

import numpy as np
import ml_dtypes

P = 128         # SBUF partitions / window size
DIM = 512
NKT = DIM // P  # 4 k-tiles
TOK = 512       # tokens per core per batch
NW = TOK // P   # 4 windows per chunk
NB = 2          # batches
NH = 8          # heads
HD = 64         # head dim
NPAIR = NH // 2
N_CORES = 8
N = 4096
SCALE = HD ** (-0.5)

W_NAMES = [f"w{p}{t}" for p in "qkv" for t in (1, 2, 3)]
O_NAMES = ["woA", "woB"]

_COMPILED = {}
LAST_RESULT = None


def _build_program(loop_n=None):
    import concourse.bacc as bacc
    import concourse.mybir as mybir
    import concourse.tile as tile
    from contextlib import ExitStack

    f32 = mybir.dt.float32
    bf16 = mybir.dt.bfloat16
    SUB = mybir.AluOpType.subtract
    ADD = mybir.AluOpType.add

    nc = bacc.Bacc(
        "TRN2",
        target_bir_lowering=False,
        debug=False,
        enable_asserts=False,
        num_devices=N_CORES,
    )

    ins = {}
    for name in ["xT_re", "xT_im", "xT_sum"]:
        ins[name] = nc.dram_tensor(name, [NB, NKT, P, TOK], bf16,
                                   kind="ExternalInput").ap()
    for name in W_NAMES:
        ins[name] = nc.dram_tensor(name, [P, NKT, DIM], bf16,
                                   kind="ExternalInput").ap()
    for name in O_NAMES:
        ins[name] = nc.dram_tensor(name, [P, NH, DIM], bf16,
                                   kind="ExternalInput").ap()
    ins["bias4"] = nc.dram_tensor("bias4", [P, NW * P], bf16,
                                  kind="ExternalInput").ap()
    ins["ident"] = nc.dram_tensor("ident", [P, P], bf16,
                                  kind="ExternalInput").ap()
    outs = {
        "y_re": nc.dram_tensor("y_re", [NB, TOK, DIM], bf16,
                               kind="ExternalOutput").ap(),
        "y_im": nc.dram_tensor("y_im", [NB, TOK, DIM], bf16,
                               kind="ExternalOutput").ap(),
    }

    with tile.TileContext(nc) as tc, ExitStack() as ctx:
        wpool = ctx.enter_context(tc.tile_pool(name="wpool", bufs=1))
        cpool = ctx.enter_context(tc.tile_pool(name="cpool", bufs=1))
        xpool = ctx.enter_context(tc.tile_pool(name="xpool", bufs=2))
        qkpool = ctx.enter_context(tc.tile_pool(name="qkpool", bufs=2))
        vpool = ctx.enter_context(tc.tile_pool(name="vpool", bufs=2))
        aopool = ctx.enter_context(tc.tile_pool(name="aopool", bufs=2))
        t3pool = ctx.enter_context(tc.tile_pool(name="t3pool", bufs=3))
        ypool = ctx.enter_context(tc.tile_pool(name="ypool", bufs=4))
        sc = ctx.enter_context(tc.tile_pool(name="sc", bufs=3))
        pp_proj = ctx.enter_context(tc.tile_pool(name="pp_proj", bufs=3,
                                                 space="PSUM"))
        pp_dots = ctx.enter_context(tc.tile_pool(name="pp_dots", bufs=3,
                                                 space="PSUM"))
        pp_tp = ctx.enter_context(tc.tile_pool(name="pp_tp", bufs=2,
                                               space="PSUM"))

        # --- resident constants -------------------------------------------
        w_sb = {}
        for name in W_NAMES:
            w_sb[name] = wpool.tile([P, NKT, DIM], bf16, name=f"sb_{name}")
        for name in O_NAMES:
            w_sb[name] = wpool.tile([P, NH, DIM], bf16, name=f"sb_{name}")

        x_sb_all = []
        for b in range(NB):
            xs = xpool.tile([P, NKT, TOK], bf16, tag="xs", name=f"xs{b}")
            xre = xpool.tile([P, NKT, TOK], bf16, tag="xre", name=f"xre{b}")
            xim = xpool.tile([P, NKT, TOK], bf16, tag="xim", name=f"xim{b}")
            x_sb_all.append((xs, xre, xim))

        # DMA queues: sync (SP) carries weights, x(b0) re/im + x(b1), y-out;
        # scalar carries xsum(b0) halves + bias/ident early (Act is idle at
        # t=0); gpsimd stays free for Pool compute (qk combines).
        # wq in pair-halves so the first q unit starts ~1us in.
        def xdma(eng, b, i, name, sl=slice(0, NKT)):
            eng.dma_start(out=x_sb_all[b][i][:, sl, :],
                          in_=ins[name][b, sl].rearrange("kt p t -> p kt t"))

        xdma(nc.scalar, 0, 0, "xT_sum", slice(0, 2))
        xdma(nc.scalar, 0, 0, "xT_sum", slice(2, 4))
        nc.sync.dma_start(out=w_sb["wq3"][:, :, 0:2 * P],
                          in_=ins["wq3"][:, :, 0:2 * P])
        nc.sync.dma_start(out=w_sb["wq1"][:, :, 0:2 * P],
                          in_=ins["wq1"][:, :, 0:2 * P])
        xdma(nc.sync, 0, 1, "xT_re", slice(0, 2))
        xdma(nc.scalar, 0, 2, "xT_im", slice(0, 2))
        nc.sync.dma_start(out=w_sb["wq2"][:, :, 0:2 * P],
                          in_=ins["wq2"][:, :, 0:2 * P])
        xdma(nc.sync, 0, 1, "xT_re", slice(2, 4))
        xdma(nc.scalar, 0, 2, "xT_im", slice(2, 4))
        bias_sb = cpool.tile([P, NW * P], bf16, name="bias_sb")
        nc.scalar.dma_start(out=bias_sb, in_=ins["bias4"])
        id_sb = cpool.tile([P, P], bf16, name="id_sb")
        nc.scalar.dma_start(out=id_sb, in_=ins["ident"])
        for t in (3, 1, 2):
            nc.sync.dma_start(out=w_sb[f"wq{t}"][:, :, 2 * P:],
                              in_=ins[f"wq{t}"][:, :, 2 * P:])
        for t in (3, 1, 2):
            nc.sync.dma_start(out=w_sb[f"wk{t}"], in_=ins[f"wk{t}"])
        for t in (3, 1, 2):
            nc.sync.dma_start(out=w_sb[f"wv{t}"], in_=ins[f"wv{t}"])
        for i, name in ((0, "xT_sum"), (1, "xT_re"), (2, "xT_im")):
            xdma(nc.sync, 1, i, name)
        for name in O_NAMES:
            nc.sync.dma_start(out=w_sb[name], in_=ins[name])

        def body():
            st = {}

            def proj_qk_unit(b, key, wp, pb):
                """One head-pair of q or k: 12 MMs + t3 copy + 4 half TTs."""
                xs, xre, xim = x_sb_all[b]
                dst = st[b][key]
                h0, h1 = 2 * pb, 2 * pb + 1
                csl = slice(pb * P, (pb + 1) * P)

                def run():
                    # All three t-terms drain PSUM->SBUF bf16 (t3 on Act,
                    # t1/t2 on DVE), then 4 half-width all-SBUF bf16 TTs
                    # (DVE 4x mode) assemble the interleaved qcat/kcat.
                    ps3 = pp_proj.tile([P, TOK], f32, tag="proj",
                                       name=f"p3{wp}{b}_{pb}")
                    for kt in range(NKT):
                        nc.tensor.matmul(ps3, w_sb[f"{wp}3"][:, kt, csl],
                                         xs[:, kt, :],
                                         start=(kt == 0), stop=(kt == 3))
                    t3 = t3pool.tile([P, TOK], bf16, tag="t3",
                                     name=f"t3{wp}{b}_{pb}")
                    nc.scalar.copy(out=t3, in_=ps3)
                    ps1 = pp_proj.tile([P, TOK], f32, tag="proj",
                                       name=f"p1{wp}{b}_{pb}")
                    for kt in range(NKT):
                        nc.tensor.matmul(ps1, w_sb[f"{wp}1"][:, kt, csl],
                                         xre[:, kt, :],
                                         start=(kt == 0), stop=(kt == 3))
                    t1 = t3pool.tile([P, TOK], bf16, tag="t1",
                                     name=f"t1{wp}{b}_{pb}")
                    nc.vector.tensor_copy(out=t1, in_=ps1)
                    ps2 = pp_proj.tile([P, TOK], f32, tag="proj",
                                       name=f"p2{wp}{b}_{pb}")
                    for kt in range(NKT):
                        nc.tensor.matmul(ps2, w_sb[f"{wp}2"][:, kt, csl],
                                         xim[:, kt, :],
                                         start=(kt == 0), stop=(kt == 3))
                    t2 = t3pool.tile([P, TOK], bf16, tag="t2",
                                     name=f"t2{wp}{b}_{pb}")
                    nc.vector.tensor_copy(out=t2, in_=ps2)
                    H = HD
                    nc.vector.tensor_tensor(out=dst[0:H, h0, :],
                                            in0=t1[0:H, :], in1=t3[0:H, :],
                                            op=SUB)
                    nc.vector.tensor_tensor(out=dst[H:P, h0, :],
                                            in0=t2[0:H, :], in1=t3[0:H, :],
                                            op=ADD)
                    nc.vector.tensor_tensor(out=dst[0:H, h1, :],
                                            in0=t1[H:P, :], in1=t3[H:P, :],
                                            op=SUB)
                    nc.vector.tensor_tensor(out=dst[H:P, h1, :],
                                            in0=t2[H:P, :], in1=t3[H:P, :],
                                            op=ADD)
                return run

            def proj_v_unit(b, it):
                """One token-block of v: 12 MMs + t3 copy + 2 strided TTs."""
                xs, xre, xim = x_sb_all[b]
                vcat = st[b]["vcat"]
                tsl = slice(it * P, (it + 1) * P)

                def run():
                    ps3 = pp_proj.tile([P, DIM], f32, tag="proj",
                                       name=f"p3v{b}_{it}")
                    for kt in range(NKT):
                        nc.tensor.matmul(ps3, xs[:, kt, tsl],
                                         w_sb["wv3"][:, kt, :],
                                         start=(kt == 0), stop=(kt == 3))
                    t3 = t3pool.tile([P, DIM], bf16, tag="t3",
                                     name=f"t3v{b}_{it}")
                    nc.scalar.copy(out=t3, in_=ps3)
                    ps1 = pp_proj.tile([P, DIM], f32, tag="proj",
                                       name=f"p1v{b}_{it}")
                    for kt in range(NKT):
                        nc.tensor.matmul(ps1, xre[:, kt, tsl],
                                         w_sb["wv1"][:, kt, :],
                                         start=(kt == 0), stop=(kt == 3))
                    ps2 = pp_proj.tile([P, DIM], f32, tag="proj",
                                       name=f"p2v{b}_{it}")
                    for kt in range(NKT):
                        nc.tensor.matmul(ps2, xim[:, kt, tsl],
                                         w_sb["wv2"][:, kt, :],
                                         start=(kt == 0), stop=(kt == 3))
                    # vcat[P, NW, NH, 2, HD]; slot 0 = re, slot 1 = im.
                    # Direct PSUM-reading TTs (one PSUM pass per output).
                    nc.vector.tensor_tensor(
                        out=vcat[:, it, :, 0, :],
                        in0=ps1.rearrange("p (h d) -> p h d", h=NH),
                        in1=t3.rearrange("p (h d) -> p h d", h=NH), op=SUB)
                    nc.vector.tensor_tensor(
                        out=vcat[:, it, :, 1, :],
                        in0=ps2.rearrange("p (h d) -> p h d", h=NH),
                        in1=t3.rearrange("p (h d) -> p h d", h=NH), op=ADD)
                return run

            def proj_qkv_units(b):
                qcat = qkpool.tile([P, NH, TOK], bf16, tag="qcat",
                                   name=f"qcat{b}")
                kcat = qkpool.tile([P, NH, TOK], bf16, tag="kcat",
                                   name=f"kcat{b}")
                vcat = vpool.tile([P, NW, NH, 2, HD], bf16, tag="vcat",
                                  name=f"vcat{b}")
                st[b] = dict(qcat=qcat, kcat=kcat, vcat=vcat)
                units = []
                for pb in range(NPAIR):
                    units.append(proj_qk_unit(b, "qcat", "wq", pb))
                for pb in range(NPAIR):
                    units.append(proj_qk_unit(b, "kcat", "wk", pb))
                for it in range(NW):
                    units.append(proj_v_unit(b, it))
                return units

            def attn_units(b):
                ao = aopool.tile([P, NH, TOK], bf16, tag="ao", name=f"ao{b}")
                st[b]["ao"] = ao
                stash = {}

                def front(h):
                    qcat, kcat = st[b]["qcat"], st[b]["kcat"]
                    pd = pp_dots.tile([P, NW, P], f32, tag="dots",
                                      name=f"pd{b}_{h}")
                    # bias preload as a PE ident-matmul (frees Act time)
                    nc.tensor.matmul(
                        pd.rearrange("p w j -> p (w j)"), id_sb, bias_sb,
                        start=True, stop=True, skip_group_check=True)
                    for w in range(NW):
                        nc.tensor.matmul(
                            pd[:, w, :],
                            qcat[:, h, w * P:(w + 1) * P],
                            kcat[:, h, w * P:(w + 1) * P],
                            start=False, stop=True, skip_group_check=True,
                        )
                    e = sc.tile([P, NW, P], bf16, tag="e", name=f"e{b}_{h}",
                                bufs=3)
                    nc.scalar.activation(out=e, in_=pd,
                                         func=mybir.ActivationFunctionType.Exp)
                    s = sc.tile([P, NW], f32, tag="s", name=f"s{b}_{h}", bufs=4)
                    nc.vector.tensor_reduce(out=s, in_=e,
                                            axis=mybir.AxisListType.X,
                                            op=mybir.AluOpType.add)
                    stash[h] = (e, s)

                def mid(h):
                    e, s = stash[h]
                    rcp = sc.tile([P, NW], f32, tag="r", name=f"r{b}_{h}",
                                  bufs=4)
                    nc.vector.reciprocal(rcp, s)
                    a = sc.tile([P, NW, P], bf16, tag="a", name=f"a{b}_{h}",
                                bufs=3)
                    for w in range(NW):
                        nc.vector.tensor_scalar_mul(a[:, w, :], e[:, w, :],
                                                    rcp[:, w:w + 1])
                    stash[h] = a

                def back_tp(h):
                    a = stash[h]
                    pt = pp_tp.tile([P, NW, P], bf16, tag="tp",
                                    name=f"pt{b}_{h}")
                    for w in range(NW):
                        nc.tensor.transpose(pt[:, w, :], a[:, w, :], id_sb)
                    at = sc.tile([P, NW, P], bf16, tag="at", name=f"at{b}_{h}",
                                 bufs=3)
                    nc.vector.tensor_copy(out=at, in_=pt)
                    stash[h] = at

                def back_pv(h):
                    at = stash.pop(h)
                    vcat = st[b]["vcat"]
                    pv = pp_dots.tile([P, NW, P], f32, tag="dots",
                                      name=f"pv{b}_{h}")
                    for w in range(NW):
                        nc.tensor.matmul(
                            pv[:, w, :],
                            vcat[:, w, h, :, :],
                            at[:, w, :],
                            start=True, stop=True,
                        )
                    nc.scalar.copy(out=st[b]["ao"][:, h, :],
                                   in_=pv.rearrange("p w j -> p (w j)"))

                return front, mid, back_tp, back_pv

            def oproj_unit(b, it, comp):
                ao = st[b]["ao"]
                tsl = slice(it * P, (it + 1) * P)
                oname = ("y_re", "y_im")[comp]
                wname = O_NAMES[comp]
                deng = (nc.sync, nc.gpsimd)[(2 * it + comp) % 2]

                def run():
                    ps = pp_proj.tile([P, DIM], f32, tag="proj",
                                      name=f"ps_{oname}{b}{it}")
                    for h in range(NH):
                        nc.tensor.matmul(ps, ao[:, h, tsl],
                                         w_sb[wname][:, h, :],
                                         start=(h == 0), stop=(h == 7))
                    ys = ypool.tile([P, DIM], bf16, tag="y",
                                    name=f"ys_{oname}{b}{it}")
                    nc.scalar.copy(out=ys[:, 0:DIM // 2], in_=ps[:, 0:DIM // 2])
                    nc.vector.tensor_copy(out=ys[:, DIM // 2:],
                                          in_=ps[:, DIM // 2:])
                    deng.dma_start(out=outs[oname][b, tsl, :], in_=ys)
                return run

            # --------------- schedule -----------------------------------
            for u in proj_qkv_units(0):
                u()
            # phase 2: attn b0 braided with qkv proj b1 (12 units / 8 heads)
            f0, m0, t0, p0 = attn_units(0)
            pb1 = proj_qkv_units(1)
            take = [2, 1, 2, 1, 2, 1, 2, 1]
            pos = 0
            for h in range(NH + 2):
                if h < NH:
                    f0(h)
                    for u in pb1[pos:pos + take[h]]:
                        u()
                    pos += take[h]
                if 1 <= h < NH + 1:
                    m0(h - 1)
                    t0(h - 1)
                if h >= 2:
                    p0(h - 2)
            # phase 3: attn b1 braided with oproj b0 (8 units / 8 heads)
            f1, m1, t1, p1 = attn_units(1)
            ob0 = [oproj_unit(0, it, c) for it in range(NW) for c in range(2)]
            ob1 = [oproj_unit(1, it, c) for it in range(NW) for c in range(2)]
            for h in range(NH + 2):
                if h < NH:
                    f1(h)
                    ob0[h]()
                if 1 <= h < NH + 1:
                    m1(h - 1)
                    t1(h - 1)
                if h >= 2:
                    p1(h - 2)
                if h == NH + 1:
                    ob1[0]()  # overlap phase-3 pipeline drain
            # phase 4: oproj b1
            for u in ob1[1:]:
                u()

        if loop_n:
            with tc.For_i(0, loop_n):
                body()
        else:
            body()

    nc.compile()
    return nc


def get_compiled(loop_n=None):
    key = loop_n
    if key not in _COMPILED:
        _COMPILED[key] = _build_program(loop_n)
    return _COMPILED[key]


def make_in_maps(x_re, x_im, wq_re, wq_im, wk_re, wk_im, wv_re, wv_im,
                 wo_re, wo_im, rel_bias):
    """Host-side prep: bf16 casts, Karatsuba weight trios, token sharding."""
    f32 = np.float32
    bf16 = ml_dtypes.bfloat16

    def swz(W):
        # [DIM(k or f), DIM] -> [P, NKT, DIM] partition-major
        return np.ascontiguousarray(
            W.reshape(NKT, P, DIM).transpose(1, 0, 2).astype(bf16))

    def trio(prefix, wre, wim, transpose, scale=1.0):
        # transpose=True: A = wre.T (qkv: y = x@W.T); False: A = wre (o: rows f')
        A = np.asarray(wre, f32) * scale
        B = np.asarray(wim, f32) * scale
        if transpose:
            A, B = A.T, B.T
        A = np.ascontiguousarray(A)
        B = np.ascontiguousarray(B)
        return {f"{prefix}1": swz(A + B), f"{prefix}2": swz(A - B),
                f"{prefix}3": swz(B)}

    shared = {}
    shared.update(trio("wq", wq_re, wq_im, True, SCALE))
    shared.update(trio("wk", wk_re, wk_im, True))
    shared.update(trio("wv", wv_re, wv_im, True))

    # stacked o-projection weights, rows f' = (h, comp, d) interleaved
    wor = np.asarray(wo_re, f32)
    woi = np.asarray(wo_im, f32)
    A = np.empty((NH * P, DIM), f32)
    B = np.empty((NH * P, DIM), f32)
    for h in range(NH):
        A[h * P:h * P + HD, :] = wor[:, h * HD:(h + 1) * HD].T
        A[h * P + HD:(h + 1) * P, :] = -woi[:, h * HD:(h + 1) * HD].T
        B[h * P:h * P + HD, :] = woi[:, h * HD:(h + 1) * HD].T
        B[h * P + HD:(h + 1) * P, :] = wor[:, h * HD:(h + 1) * HD].T
    def swz_o(W):
        return np.ascontiguousarray(
            W.reshape(NH, P, DIM).transpose(1, 0, 2).astype(bf16))
    shared["woA"] = swz_o(A)
    shared["woB"] = swz_o(B)

    idx = np.arange(P)[None, :] - np.arange(P)[:, None] + P
    bias_mat = np.asarray(rel_bias, f32)[idx]
    shared["bias4"] = np.ascontiguousarray(
        np.tile(bias_mat, (1, NW))).astype(bf16)
    shared["ident"] = np.eye(P, dtype=bf16)

    x_re = np.asarray(x_re, f32)
    x_im = np.asarray(x_im, f32)
    x_sum = x_re + x_im
    in_maps = []
    for c in range(N_CORES):
        sl = slice(c * TOK, (c + 1) * TOK)
        m = dict(shared)
        for name, arr in (("xT_re", x_re), ("xT_im", x_im),
                          ("xT_sum", x_sum)):
            m[name] = np.ascontiguousarray(
                arr[:, sl, :].transpose(0, 2, 1).reshape(NB, NKT, P, TOK)
                .astype(bf16))
        in_maps.append(m)
    return in_maps


def assemble_output(results):
    out = np.empty((2, NB, N, DIM), np.float32)
    for c in range(N_CORES):
        sl = slice(c * TOK, (c + 1) * TOK)
        out[0, :, sl, :] = results[c]["y_re"].astype(np.float32)
        out[1, :, sl, :] = results[c]["y_im"].astype(np.float32)
    return out


def kernel(**inputs):
    global LAST_RESULT
    import os
    from concourse.bass_utils import run_bass_kernel_spmd

    nc = get_compiled()
    in_maps = make_in_maps(**inputs)
    core_ids = list(range(N_CORES))
    try:
        res = run_bass_kernel_spmd(nc, in_maps, core_ids)
    except ModuleNotFoundError:
        os.environ["BASS_NEVER_TRACE"] = "1"
        res = run_bass_kernel_spmd(nc, in_maps, core_ids)
    LAST_RESULT = res
    return assemble_output(res.results)
